# revision 27
# baseline (speedup 1.0000x reference)
"""Trainium2 Bass kernel for GPT-2 style attention block (B=2, S=2048, D=1024, H=16).

Sharding (8 cores): data-parallel over batch (2) x tensor-parallel over heads (4 per
core). Each core: QKV projection for its 4 heads over the full sequence, full-seq
causal attention (transposed-scores layout: softmax reduction folded into the PV
matmul via a ones-column in V), then a row-parallel partial c_proj over the full
sequence using only this core's 256 rows of c_proj_w. No collectives: the host
sums the 4 per-head-group partials per batch (plus the folded v-bias term), so
each core's span is pure compute with no cross-core sync.

Schedule: work is organized in per-qt rounds (512 query columns each). The q/k
projections for query-block qt and key-block qt plus the V pieces for that key
range are emitted inside the round, so the tensor engine always has dense matmul
work to overlap the softmax exps (scalar engine) and stays HAM-warm.

Causal structure: score blocks on the diagonal are shrunk to their unmasked
query range and packed contiguously in PSUM so the exp covers no dead columns.
Masking is post-exp: GpSimd multiplies the probability tile by a 0/1 triangle
in SBUF (keeps DVE and the PSUM ports free). Softmax normalization is off-PE:
reciprocal on DVE, partition broadcast on GpSimd, fused scale-multiply on DVE.

Compute dtype bf16 (fp32 PSUM accumulation); normalization in fp32; partial
outputs shipped as fp16 to halve DMA.
"""
import sys
sys.path.insert(0, '/opt/trn_rl_repo')

import numpy as np
import ml_dtypes

import concourse.bass as bass
import concourse.mybir as mybir
import concourse.tile as tile
from concourse import bacc
from concourse.bass_utils import run_bass_kernel_spmd

B, S, D = 2, 2048, 1024
H, HD = 16, 64
NCORES = 8
HPC = H // 4          # heads per core = 4

F32 = mybir.dt.float32
F16 = mybir.dt.float16
BF16 = mybir.dt.bfloat16
ADD = mybir.AluOpType.add
MULT = mybir.AluOpType.mult
EXP = mybir.ActivationFunctionType.Exp


def _emit(nc, tc):
    xT = nc.dram_tensor("xT", [D, S], BF16, kind="ExternalInput").ap()
    w_qk = nc.dram_tensor("w_qk", [D, 512], BF16, kind="ExternalInput").ap()
    w_v = nc.dram_tensor("w_v", [D, 256], BF16, kind="ExternalInput").ap()
    w_p = nc.dram_tensor("w_p", [256, D], BF16, kind="ExternalInput").ap()
    bqk = nc.dram_tensor("bqk", [128, 4], F32, kind="ExternalInput").ap()
    cmask = nc.dram_tensor("cmask", [128, 128], BF16, kind="ExternalInput").ap()
    out = nc.dram_tensor("out", [S, D], F16, kind="ExternalOutput").ap()

    from contextlib import ExitStack
    ctx = ExitStack()
    cst = ctx.enter_context(tc.tile_pool(name="cst", bufs=1))
    pw = ctx.enter_context(tc.tile_pool(name="pw", bufs=2, space="PSUM"))
    pat = ctx.enter_context(tc.tile_pool(name="pat", bufs=2, space="PSUM"))
    psc = ctx.enter_context(tc.tile_pool(name="psc", bufs=2, space="PSUM"))
    sb = ctx.enter_context(tc.tile_pool(name="sb", bufs=4))

    # ---- resident SBUF loads, split across both HWDGE queues (sync + scalar)
    # and ordered so round 0's operands land first ----
    xT_sb = cst.tile([128, 8, S], BF16)
    wqk_sb = cst.tile([128, 8, 512], BF16)
    wv_sb = cst.tile([128, 8, 256], BF16)
    bqk_sb = cst.tile([128, 4], F32)
    cm_sb = cst.tile([128, 128], BF16)
    wp_sb = cst.tile([128, 2, D], BF16)
    xT_r = xT.rearrange("(k p) n -> p k n", p=128)
    wqk_r = w_qk.rearrange("(k p) n -> p k n", p=128)
    wv_r = w_v.rearrange("(k p) n -> p k n", p=128)
    nc.scalar.dma_start(bqk_sb[:], bqk)
    nc.scalar.dma_start(cm_sb[:], cmask)
    # round-0-critical prefix (xT qt0 + wqk) split across both queues
    for k in range(0, 8, 2):
        nc.sync.dma_start(xT_sb[:, k, 0:512], xT_r[:, k, 0:512])
        nc.sync.dma_start(wqk_sb[:, k], wqk_r[:, k])
        nc.scalar.dma_start(xT_sb[:, k + 1, 0:512], xT_r[:, k + 1, 0:512])
        nc.scalar.dma_start(wqk_sb[:, k + 1], wqk_r[:, k + 1])
    # rest: xT on sync (qt-major), v/proj weights on scalar
    for qt in range(1, 4):
        for k in range(8):
            nc.sync.dma_start(
                xT_sb[:, k, qt * 512:(qt + 1) * 512],
                xT_r[:, k, qt * 512:(qt + 1) * 512])
    for k in range(8):
        nc.scalar.dma_start(wv_sb[:, k], wv_r[:, k])
    nc.scalar.dma_start(wp_sb[:], w_p.rearrange("(k p) n -> p k n", p=128))

    # PE warmer: dependency-free junk matmuls keep the array busy during the
    # input DMAs so HAM unthrottles before real work arrives
    ones_sb = cst.tile([1, 64], BF16)
    nc.vector.memset(ones_sb[:], 1.0)
    wrow = sb.tile([1, 512], BF16, tag="wrow")
    nc.vector.memset(wrow[:], 1.0)
    warm_ps = pw.tile([128, 512], F32, tag="w", name="warm")
    for _ in range(12):
        nc.tensor.matmul(warm_ps[0:64, :], ones_sb[:], wrow[:],
                         start=True, stop=True)

    # qkT [512, 2048]: rows 0-255 = q^T (4 heads x 64, prescaled 1/8), 256-511 = k^T
    qkT_sb = cst.tile([128, 4, S], BF16)

    def qk_proj(m, qt):
        # q^T (m=0,1) / k^T (m=2,3) for one 512-column sequence block
        ps = pw.tile([128, 512], F32, tag="w", name=f"qk{m}_{qt}")
        for k in range(8):
            nc.tensor.matmul(
                ps[:], wqk_sb[:, k, m * 128:(m + 1) * 128],
                xT_sb[:, k, qt * 512:(qt + 1) * 512],
                start=(k == 0), stop=(k == 7))
        nc.vector.tensor_scalar(
            out=qkT_sb[:, m, qt * 512:(qt + 1) * 512], in0=ps[:],
            scalar1=bqk_sb[:, m:m + 1], scalar2=None, op0=ADD)

    # V with interleaved ones column: V_sb [128, 16, 4*65]
    V_sb = cst.tile([128, 16, HPC * 65], BF16)

    def v_ones():
        nc.vector.memset(
            V_sb[:].rearrange("p m (h c) -> p m h c", c=65)[:, :, :, 64:65], 1.0)

    def v_piece(m):
        ps = pw.tile([128, 512], F32, tag="w", name=f"v{m}")
        for k in range(8):
            nc.tensor.matmul(
                ps[:, :256], xT_sb[:, k, m * 128:(m + 1) * 128], wv_sb[:, k, :],
                start=(k == 0), stop=(k == 7))
        nc.vector.tensor_copy(
            out=V_sb[:, m].rearrange("p (h c) -> p h c", c=65)[:, :, 0:64],
            in_=ps[:, :256].rearrange("p (h c) -> p h c", c=64))

    attnT_sb = cst.tile([128, 2, S], BF16)

    def attend_pair(j, qt, defer_v=(), fillers=None):
        # heads 2j (partitions 0-63) and 2j+1 (64-127) interleaved: their K=64
        # score matmuls auto-derive different PE row-groups from base_partition
        # and run concurrently when adjacent in the queue. One key block per
        # step, both heads side by side in one PSUM tile -> one exp call covers
        # the pair and psc stays double-buffered. Head A sits at column 0,
        # head B at column 512: each matmul output must stay inside one PSUM
        # bank, so B's offset is fixed even when the diagonal block is shrunk.
        sub = j
        at = {0: pat.tile([128, 512], F32, tag="at", name=f"atA{j}_{qt}"),
              64: pat.tile([128, 512], F32, tag="at", name=f"atB{j}_{qt}")}
        nkb = 4 * qt + 4

        def scores_kb(kb):
            rel = max(0, kb * 128 - qt * 512)
            wb = 512 - rel
            sc = psc.tile([128, 1024], F32, tag="sc", name=f"sc{j}_{qt}_{kb}")
            for i, po in enumerate((0, 64)):
                nc.tensor.matmul(
                    sc[:, i * 512:i * 512 + wb],
                    qkT_sb[po:po + 64, 2 + sub, kb * 128:(kb + 1) * 128],
                    qkT_sb[po:po + 64, sub, qt * 512 + rel:(qt + 1) * 512],
                    start=True, stop=True)
            pt = sb.tile([128, 1024], BF16, tag="pt", name=f"pt{j}_{qt}_{kb}")
            nc.scalar.activation(out=pt[:, :512 + wb], in_=sc[:, :512 + wb],
                                 func=EXP)
            if kb * 128 >= qt * 512:  # post-exp triangle zeroing, both heads
                for i in (0, 1):
                    nc.vector.tensor_tensor(
                        pt[:, i * 512:i * 512 + 128],
                        pt[:, i * 512:i * 512 + 128], cm_sb[:], MULT)
            return pt

        def pv_kb(kb, pt):
            rel = max(0, kb * 128 - qt * 512)
            wb = 512 - rel
            for i, po in enumerate((0, 64)):
                h = 2 * j + i
                nc.tensor.matmul(
                    at[po][0:65, rel:512], V_sb[:, kb, h * 65:(h + 1) * 65],
                    pt[:, i * 512:i * 512 + wb],
                    start=(kb == 0), stop=(kb == nkb - 1))

        if defer_v:
            # round 0 only: all scores/exps go first so the first exps aren't
            # queued behind 32 cold-start V matmuls; V lands before the PVs
            pts = [scores_kb(kb) for kb in range(nkb)]
            for m in defer_v:
                v_piece(m)
            for kb in range(nkb):
                pv_kb(kb, pts[kb])
        else:
            # thread filler projection work into the exp-paced kb loop so the
            # tensor engine stays dense (and HAM-warm) while ACT chews exps
            for kb in range(nkb):
                pv_kb(kb, scores_kb(kb))
                if fillers and kb % 2 == 1:
                    fillers.pop(0)()
        # quick-release the at banks (2 fast DVE copies each), then normalize
        # off the critical path: both heads' denominators packed in one row,
        # one reciprocal (DVE), one partition broadcast (GpSimd), and two
        # partition-aligned in-place multiplies on SBUF (DVE)
        den2 = sb.tile([1, 1024], F32, tag="den2")
        for po in (0, 64):
            sl = attnT_sb[po:po + 64, sub, qt * 512:(qt + 1) * 512]
            nc.vector.tensor_copy(out=sl, in_=at[po][0:64, :])
            nc.vector.tensor_copy(out=den2[:, po * 8:po * 8 + 512],
                                  in_=at[po][64:65, :])
        rec2 = sb.tile([1, 1024], F32, tag="rec2")
        nc.vector.reciprocal_approx_fast(rec2[:], den2[:])
        recb = sb.tile([128, 1024], F32, tag="recb")
        nc.gpsimd.partition_broadcast(recb[:], rec2[:])
        for po in (0, 64):
            sl = attnT_sb[po:po + 64, sub, qt * 512:(qt + 1) * 512]
            nc.vector.tensor_tensor(
                sl, sl, recb[po:po + 64, po * 8:po * 8 + 512], MULT)

    def c_proj(ms, tail=False):
        # partial c_proj: contract only this core's 256 D-rows (2 u-blocks of
        # 128), full 2048-seq output; host sums partials across head groups.
        # In the tail (scalar engine idle) the PSUM evacuation is split across
        # Vector and Scalar and the out-DMA goes on the second queue.
        for m in ms:
            out_sb = sb.tile([128, D], F16, tag="out")
            ps = [pw.tile([128, 512], F32, tag="w", name=f"pj{m}_{n}") for n in range(2)]
            for u in range(2):
                for n in range(2):
                    nc.tensor.matmul(
                        ps[n][:], attnT_sb[:, u, m * 128:(m + 1) * 128],
                        wp_sb[:, u, n * 512:(n + 1) * 512],
                        start=(u == 0), stop=(u == 1))
            nc.vector.tensor_copy(out=out_sb[:, 0:512], in_=ps[0][:])
            if tail:
                nc.scalar.activation(out=out_sb[:, 512:1024], in_=ps[1][:],
                                     func=mybir.ActivationFunctionType.Copy)
                nc.scalar.dma_start(out[m * 128:(m + 1) * 128, :], out_sb[:])
            else:
                nc.vector.tensor_copy(out=out_sb[:, 512:1024], in_=ps[1][:])
                nc.sync.dma_start(out[m * 128:(m + 1) * 128, :], out_sb[:])

    # ---- per-qt rounds, software-pipelined: the next round's q/k projections
    # and V pieces are emitted before the current round ends so the scalar
    # engine's exp stream never drains at round boundaries; prev-round c_proj
    # blocks fill exp-gated stretches ----
    v_ones()
    qk_proj(0, 0)
    qk_proj(2, 0)
    attend_pair(0, 0, defer_v=(0, 1, 2, 3))
    qk_proj(1, 0)
    qk_proj(3, 0)
    attend_pair(1, 0)
    qk_proj(0, 1)
    qk_proj(2, 1)
    for m in range(4, 8):
        v_piece(m)
    for qt in range(1, 4):
        # fillers drain inside the kb loops; leftovers are emitted right after,
        # always before any consumer pair
        f0 = [lambda m=m: qk_proj(m, qt) for m in (1, 3)]
        f0 += [lambda m=m: c_proj((m,)) for m in (4 * qt - 4, 4 * qt - 3)]
        attend_pair(0, qt, fillers=f0)
        for fn in f0:
            fn()
        f1 = []
        if qt < 3:
            f1 += [lambda m=m, q=qt + 1: qk_proj(m, q) for m in (0, 2)]
            f1 += [lambda m=m: v_piece(m) for m in range(4 * qt + 4, 4 * qt + 8)]
        f1 += [lambda m=m: c_proj((m,)) for m in (4 * qt - 2, 4 * qt - 1)]
        attend_pair(1, qt, fillers=f1)
        for fn in f1:
            fn()
    c_proj(tuple(range(12, 16)), tail=True)

    ctx.close()


def build_nc():
    nc = bacc.Bacc("TRN2", target_bir_lowering=False, debug=False, num_devices=NCORES)
    with tile.TileContext(nc) as tc:
        _emit(nc, tc)
    nc.compile()
    return nc


def shard_inputs(hidden_states, c_attn_w, c_attn_b, c_proj_w, c_proj_b):
    x = np.asarray(hidden_states, np.float32)
    W = np.asarray(c_attn_w, np.float32)
    bqkv = np.asarray(c_attn_b, np.float32)
    Wp = np.asarray(c_proj_w, np.float32)

    wq, wk, wv = W[:, :D] * 0.125, W[:, D:2 * D], W[:, 2 * D:]
    bq, bk = bqkv[:D] * 0.125, bqkv[D:2 * D]

    # 128x128 causal triangle keep-mask: 0 where key (row) > query (col), else 1
    k_i = np.arange(128)[:, None]
    q_i = np.arange(128)[None, :]
    cm = (k_i <= q_i).astype(ml_dtypes.bfloat16)

    in_maps = []
    for c in range(NCORES):
        b, r = divmod(c, 4)
        hs = slice(256 * r, 256 * (r + 1))
        w_qk = np.concatenate([wq[:, hs], wk[:, hs]], axis=1)
        bqk_t = np.concatenate([bq[hs], bk[hs]]).reshape(4, 128).T.copy()
        in_maps.append(dict(
            xT=np.ascontiguousarray(x[b].T).astype(ml_dtypes.bfloat16),
            w_qk=w_qk.astype(ml_dtypes.bfloat16),
            w_v=wv[:, hs].astype(ml_dtypes.bfloat16),
            w_p=np.ascontiguousarray(Wp[hs, :]).astype(ml_dtypes.bfloat16),
            bqk=bqk_t.astype(np.float32),
            cmask=cm,
        ))
    return in_maps


def unshard(results, c_attn_b, c_proj_w, c_proj_b):
    bqkv = np.asarray(c_attn_b, np.float32)
    Wp = np.asarray(c_proj_w, np.float32)
    bp = np.asarray(c_proj_b, np.float32)
    # softmax rows sum to 1, so the v-bias passes through attention unchanged:
    # out = (softmax @ xWv + bv) @ Wp + bp = sum(partials) + bv@Wp + bp
    beff = (bqkv[2 * D:] @ Wp + bp).astype(np.float32)
    full = np.zeros((B, S, D), np.float32)
    for c in range(NCORES):
        b = c // 4
        full[b] += results[c]["out"].astype(np.float32)
    full += beff
    return full


_NC = None


def kernel(**inputs):
    global _NC
    if _NC is None:
        _NC = build_nc()
    in_maps = shard_inputs(**inputs)
    res = run_bass_kernel_spmd(_NC, in_maps, core_ids=list(range(NCORES)))
    return unshard(res.results, inputs["c_attn_b"], inputs["c_proj_w"],
                   inputs["c_proj_b"])


if __name__ == "__main__":
    import jax
    with jax.default_device(jax.devices("cpu")[0]):
        import reference
        inputs = {k: np.asarray(v) for k, v in reference.setup_inputs().items()}
        expected = np.asarray(reference.reference(**inputs))
    actual = kernel(**inputs)
    err = np.abs(actual - expected)
    print("max abs err:", err.max(), "rel:", err.max() / np.abs(expected).max())


# revision 32
# speedup vs baseline: 1.0279x; 1.0279x over previous
"""Trainium2 Bass kernel for GPT-2 style attention block (B=2, S=2048, D=1024, H=16).

Sharding (8 cores): data-parallel over batch (2) x tensor-parallel over heads (4 per
core). Each core: QKV projection for its 4 heads over the full sequence, full-seq
causal attention (transposed-scores layout: softmax reduction folded into the PV
matmul via a ones-column in V), then a row-parallel partial c_proj over the full
sequence using only this core's 256 rows of c_proj_w. No collectives: the host
sums the 4 per-head-group partials per batch (plus the folded v-bias term), so
each core's span is pure compute with no cross-core sync.

Schedule: work is organized in per-qt rounds (512 query columns each). The q/k
projections for query-block qt and key-block qt plus the V pieces for that key
range are emitted inside the round, so the tensor engine always has dense matmul
work to overlap the softmax exps (scalar engine) and stays HAM-warm.

Causal structure: score blocks on the diagonal are shrunk to their unmasked
query range and packed contiguously in PSUM so the exp covers no dead columns.
Masking is post-exp: GpSimd multiplies the probability tile by a 0/1 triangle
in SBUF (keeps DVE and the PSUM ports free). Softmax normalization is off-PE:
reciprocal on DVE, partition broadcast on GpSimd, fused scale-multiply on DVE.

Compute dtype bf16 (fp32 PSUM accumulation); normalization in fp32; partial
outputs shipped as fp16 to halve DMA.
"""
import sys
sys.path.insert(0, '/opt/trn_rl_repo')

import numpy as np
import ml_dtypes

import concourse.bass as bass
import concourse.mybir as mybir
import concourse.tile as tile
from concourse import bacc
from concourse.bass_utils import run_bass_kernel_spmd

B, S, D = 2, 2048, 1024
H, HD = 16, 64
NCORES = 8
HPC = H // 4          # heads per core = 4

F32 = mybir.dt.float32
F16 = mybir.dt.float16
BF16 = mybir.dt.bfloat16
ADD = mybir.AluOpType.add
MULT = mybir.AluOpType.mult
EXP = mybir.ActivationFunctionType.Exp


def _emit(nc, tc):
    xT = nc.dram_tensor("xT", [D, S], BF16, kind="ExternalInput").ap()
    w_qk = nc.dram_tensor("w_qk", [D, 512], BF16, kind="ExternalInput").ap()
    w_v = nc.dram_tensor("w_v", [D, 256], BF16, kind="ExternalInput").ap()
    w_p = nc.dram_tensor("w_p", [256, D], BF16, kind="ExternalInput").ap()
    bqk = nc.dram_tensor("bqk", [128, 4], F32, kind="ExternalInput").ap()
    cmask = nc.dram_tensor("cmask", [128, 128], BF16, kind="ExternalInput").ap()
    out = nc.dram_tensor("out", [S, D], F16, kind="ExternalOutput").ap()

    from contextlib import ExitStack
    ctx = ExitStack()
    cst = ctx.enter_context(tc.tile_pool(name="cst", bufs=1))
    pw = ctx.enter_context(tc.tile_pool(name="pw", bufs=2, space="PSUM"))
    pat = ctx.enter_context(tc.tile_pool(name="pat", bufs=2, space="PSUM"))
    psc = ctx.enter_context(tc.tile_pool(name="psc", bufs=2, space="PSUM"))
    sb = ctx.enter_context(tc.tile_pool(name="sb", bufs=4))

    # ---- resident SBUF loads, split across both HWDGE queues (sync + scalar)
    # and ordered so round 0's operands land first ----
    xT_sb = cst.tile([128, 8, S], BF16)
    wqk_sb = cst.tile([128, 8, 512], BF16)
    wv_sb = cst.tile([128, 8, 256], BF16)
    bqk_sb = cst.tile([128, 4], F32)
    cm_sb = cst.tile([128, 128], BF16)
    wp_sb = cst.tile([128, 2, D], BF16)
    xT_r = xT.rearrange("(k p) n -> p k n", p=128)
    wqk_r = w_qk.rearrange("(k p) n -> p k n", p=128)
    wv_r = w_v.rearrange("(k p) n -> p k n", p=128)
    nc.scalar.dma_start(bqk_sb[:], bqk)
    nc.scalar.dma_start(cm_sb[:], cmask)
    # round-0-critical prefix (xT qt0 + wqk) split across both queues
    for k in range(0, 8, 2):
        nc.sync.dma_start(xT_sb[:, k, 0:512], xT_r[:, k, 0:512])
        nc.sync.dma_start(wqk_sb[:, k], wqk_r[:, k])
        nc.scalar.dma_start(xT_sb[:, k + 1, 0:512], xT_r[:, k + 1, 0:512])
        nc.scalar.dma_start(wqk_sb[:, k + 1], wqk_r[:, k + 1])
    # rest: xT on sync (qt-major), v/proj weights on scalar
    for qt in range(1, 4):
        for k in range(8):
            nc.sync.dma_start(
                xT_sb[:, k, qt * 512:(qt + 1) * 512],
                xT_r[:, k, qt * 512:(qt + 1) * 512])
    for k in range(8):
        nc.scalar.dma_start(wv_sb[:, k], wv_r[:, k])
    nc.scalar.dma_start(wp_sb[:], w_p.rearrange("(k p) n -> p k n", p=128))

    # PE warmer: dependency-free junk matmuls keep the array busy during the
    # input DMAs so HAM unthrottles before real work arrives
    ones_sb = cst.tile([1, 128], BF16)
    nc.vector.memset(ones_sb[:], 1.0)
    wrow = sb.tile([1, 512], BF16, tag="wrow")
    nc.vector.memset(wrow[:], 1.0)
    warm_ps = pw.tile([128, 512], F32, tag="w", name="warm")
    for _ in range(12):
        nc.tensor.matmul(warm_ps[:], ones_sb[:], wrow[:],
                         start=True, stop=True)

    # qkT [512, 2048]: rows 0-255 = q^T (4 heads x 64, prescaled 1/8), 256-511 = k^T
    qkT_sb = cst.tile([128, 4, S], BF16)

    def qk_proj(m, qt):
        # q^T (m=0,1) / k^T (m=2,3) for one 512-column sequence block
        ps = pw.tile([128, 512], F32, tag="w", name=f"qk{m}_{qt}")
        for k in range(8):
            nc.tensor.matmul(
                ps[:], wqk_sb[:, k, m * 128:(m + 1) * 128],
                xT_sb[:, k, qt * 512:(qt + 1) * 512],
                start=(k == 0), stop=(k == 7))
        nc.vector.tensor_scalar(
            out=qkT_sb[:, m, qt * 512:(qt + 1) * 512], in0=ps[:],
            scalar1=bqk_sb[:, m:m + 1], scalar2=None, op0=ADD)

    # V with interleaved ones column: V_sb [128, 16, 4*65]
    V_sb = cst.tile([128, 16, HPC * 65], BF16)

    def v_ones():
        nc.vector.memset(
            V_sb[:].rearrange("p m (h c) -> p m h c", c=65)[:, :, :, 64:65], 1.0)

    def v_piece(m):
        ps = pw.tile([128, 512], F32, tag="w", name=f"v{m}")
        for k in range(8):
            nc.tensor.matmul(
                ps[:, :256], xT_sb[:, k, m * 128:(m + 1) * 128], wv_sb[:, k, :],
                start=(k == 0), stop=(k == 7))
        nc.vector.tensor_copy(
            out=V_sb[:, m].rearrange("p (h c) -> p h c", c=65)[:, :, 0:64],
            in_=ps[:, :256].rearrange("p (h c) -> p h c", c=64))

    attnT_sb = cst.tile([128, 2, S], BF16)

    def attend_pair(j, qt, defer_v=(), fillers=None, fast_norm=False):
        # heads 2j (partitions 0-63) and 2j+1 (64-127) interleaved: their K=64
        # score matmuls auto-derive different PE row-groups from base_partition
        # and run concurrently when adjacent in the queue. One key block per
        # step, both heads side by side in one PSUM tile -> one exp call covers
        # the pair and psc stays double-buffered. Head A sits at column 0,
        # head B at column 512: each matmul output must stay inside one PSUM
        # bank, so B's offset is fixed even when the diagonal block is shrunk.
        sub = j
        at = {0: pat.tile([128, 512], F32, tag="at", name=f"atA{j}_{qt}"),
              64: pat.tile([128, 512], F32, tag="at", name=f"atB{j}_{qt}")}
        nkb = 4 * qt + 4

        def scores_kb(kb):
            rel = max(0, kb * 128 - qt * 512)
            wb = 512 - rel
            sc = psc.tile([128, 1024], F32, tag="sc", name=f"sc{j}_{qt}_{kb}")
            for i, po in enumerate((0, 64)):
                nc.tensor.matmul(
                    sc[:, i * 512:i * 512 + wb],
                    qkT_sb[po:po + 64, 2 + sub, kb * 128:(kb + 1) * 128],
                    qkT_sb[po:po + 64, sub, qt * 512 + rel:(qt + 1) * 512],
                    start=True, stop=True)
            pt = sb.tile([128, 1024], BF16, tag="pt", name=f"pt{j}_{qt}_{kb}")
            nc.scalar.activation(out=pt[:, :512 + wb], in_=sc[:, :512 + wb],
                                 func=EXP)
            if kb * 128 >= qt * 512:  # post-exp triangle zeroing, both heads
                for i in (0, 1):
                    nc.vector.tensor_tensor(
                        pt[:, i * 512:i * 512 + 128],
                        pt[:, i * 512:i * 512 + 128], cm_sb[:], MULT)
            return pt

        def pv_kb(kb, pt):
            rel = max(0, kb * 128 - qt * 512)
            wb = 512 - rel
            for i, po in enumerate((0, 64)):
                h = 2 * j + i
                nc.tensor.matmul(
                    at[po][0:65, rel:512], V_sb[:, kb, h * 65:(h + 1) * 65],
                    pt[:, i * 512:i * 512 + wb],
                    start=(kb == 0), stop=(kb == nkb - 1))

        if defer_v:
            # round 0 only: all scores/exps go first so the first exps aren't
            # queued behind 32 cold-start V matmuls; V lands before the PVs
            pts = [scores_kb(kb) for kb in range(nkb)]
            for m in defer_v:
                v_piece(m)
            for kb in range(nkb):
                pv_kb(kb, pts[kb])
        else:
            # thread filler projection work into the exp-paced kb loop so the
            # tensor engine stays dense (and HAM-warm) while ACT chews exps
            for kb in range(nkb):
                pv_kb(kb, scores_kb(kb))
                if fillers and kb % 2 == 1:
                    fillers.pop(0)()
        # quick-release the at banks (2 fast DVE copies each), then normalize
        # off the critical path: both heads' denominators packed in one row,
        # one reciprocal (DVE), one partition broadcast (GpSimd), and two
        # partition-aligned in-place multiplies on SBUF (DVE). fast_norm (for
        # the final pair, where the PE is idle and the c_proj tail waits on
        # attnT) broadcasts via a low-latency PE matmul instead of GpSimd.
        den2 = sb.tile([1, 1024], F32, tag="den2")
        for po in (0, 64):
            sl = attnT_sb[po:po + 64, sub, qt * 512:(qt + 1) * 512]
            nc.vector.tensor_copy(out=sl, in_=at[po][0:64, :])
            nc.vector.tensor_copy(out=den2[:, po * 8:po * 8 + 512],
                                  in_=at[po][64:65, :])
        rec2 = sb.tile([1, 1024], F32, tag="rec2")
        nc.vector.reciprocal_approx_fast(rec2[:], den2[:])
        if fast_norm:
            rec2b = sb.tile([1, 1024], BF16, tag="rec2b")
            nc.vector.tensor_copy(out=rec2b[:], in_=rec2[:])
            bc = {po: pw.tile([128, 512], F32, tag="w", name=f"bc{j}_{qt}_{po}")
                  for po in (0, 64)}
            for po in (0, 64):
                nc.tensor.matmul(bc[po][:], ones_sb[:],
                                 rec2b[:, po * 8:po * 8 + 512],
                                 start=True, stop=True)
                sl = attnT_sb[po:po + 64, sub, qt * 512:(qt + 1) * 512]
                nc.vector.tensor_tensor(
                    sl, sl, bc[po][po:po + 64, :], MULT)
        else:
            recb = sb.tile([128, 1024], F32, tag="recb")
            nc.gpsimd.partition_broadcast(recb[:], rec2[:])
            for po in (0, 64):
                sl = attnT_sb[po:po + 64, sub, qt * 512:(qt + 1) * 512]
                nc.vector.tensor_tensor(
                    sl, sl, recb[po:po + 64, po * 8:po * 8 + 512], MULT)

    def c_proj(ms, tail=False):
        # partial c_proj: contract only this core's 256 D-rows (2 u-blocks of
        # 128), full 2048-seq output; host sums partials across head groups.
        # In the tail (scalar engine idle) the PSUM evacuation is split across
        # Vector and Scalar and the out-DMA goes on the second queue.
        for m in ms:
            out_sb = sb.tile([128, D], F16, tag="out")
            ps = [pw.tile([128, 512], F32, tag="w", name=f"pj{m}_{n}") for n in range(2)]
            for u in range(2):
                for n in range(2):
                    nc.tensor.matmul(
                        ps[n][:], attnT_sb[:, u, m * 128:(m + 1) * 128],
                        wp_sb[:, u, n * 512:(n + 1) * 512],
                        start=(u == 0), stop=(u == 1))
            nc.vector.tensor_copy(out=out_sb[:, 0:512], in_=ps[0][:])
            if tail:
                nc.scalar.activation(out=out_sb[:, 512:1024], in_=ps[1][:],
                                     func=mybir.ActivationFunctionType.Copy)
                nc.scalar.dma_start(out[m * 128:(m + 1) * 128, :], out_sb[:])
            else:
                nc.vector.tensor_copy(out=out_sb[:, 512:1024], in_=ps[1][:])
                nc.sync.dma_start(out[m * 128:(m + 1) * 128, :], out_sb[:])

    # ---- per-qt rounds, software-pipelined: the next round's q/k projections
    # and V pieces are emitted before the current round ends so the scalar
    # engine's exp stream never drains at round boundaries; prev-round c_proj
    # blocks fill exp-gated stretches ----
    v_ones()
    qk_proj(0, 0)
    qk_proj(2, 0)
    attend_pair(0, 0, defer_v=(0, 1, 2, 3))
    qk_proj(1, 0)
    qk_proj(3, 0)
    attend_pair(1, 0)
    qk_proj(0, 1)
    qk_proj(2, 1)
    for m in range(4, 8):
        v_piece(m)
    # fillers drain inside the kb loops (at odd kb steps); leftovers are
    # emitted right after, always before any consumer pair. c_proj blocks are
    # weighted toward round 3 where no projection work remains, and two are
    # held past the last pair to cover its normalize latency.
    filler_plan = {
        (1, 0): [lambda: qk_proj(1, 1), lambda: qk_proj(3, 1)],
        (1, 1): [lambda: qk_proj(0, 2), lambda: qk_proj(2, 2)]
                + [lambda m=m: v_piece(m) for m in range(8, 12)]
                + [lambda: c_proj((0,)), lambda: c_proj((1,))],
        (2, 0): [lambda: qk_proj(1, 2), lambda: qk_proj(3, 2),
                 lambda: c_proj((2,)), lambda: c_proj((3,))],
        (2, 1): [lambda: qk_proj(0, 3), lambda: qk_proj(2, 3),
                 lambda: c_proj((4,)), lambda: c_proj((5,)),
                 lambda: c_proj((6,))],
        (3, 0): [lambda m=m: v_piece(m) for m in range(12, 16)]
                + [lambda: qk_proj(1, 3), lambda: qk_proj(3, 3),
                   lambda: c_proj((7,)), lambda: c_proj((8,))],
        (3, 1): [lambda: c_proj((9,)), lambda: c_proj((10,))],
    }
    for qt in range(1, 4):
        for j in (0, 1):
            f = filler_plan[(qt, j)]
            attend_pair(j, qt, fillers=f, fast_norm=(qt == 3 and j == 1))
            for fn in f:
                fn()
    c_proj((11,))
    c_proj(tuple(range(12, 16)), tail=True)

    ctx.close()


def build_nc():
    nc = bacc.Bacc("TRN2", target_bir_lowering=False, debug=False, num_devices=NCORES)
    with tile.TileContext(nc) as tc:
        _emit(nc, tc)
    nc.compile()
    return nc


def shard_inputs(hidden_states, c_attn_w, c_attn_b, c_proj_w, c_proj_b):
    x = np.asarray(hidden_states, np.float32)
    W = np.asarray(c_attn_w, np.float32)
    bqkv = np.asarray(c_attn_b, np.float32)
    Wp = np.asarray(c_proj_w, np.float32)

    wq, wk, wv = W[:, :D] * 0.125, W[:, D:2 * D], W[:, 2 * D:]
    bq, bk = bqkv[:D] * 0.125, bqkv[D:2 * D]

    # 128x128 causal triangle keep-mask: 0 where key (row) > query (col), else 1
    k_i = np.arange(128)[:, None]
    q_i = np.arange(128)[None, :]
    cm = (k_i <= q_i).astype(ml_dtypes.bfloat16)

    in_maps = []
    for c in range(NCORES):
        b, r = divmod(c, 4)
        hs = slice(256 * r, 256 * (r + 1))
        w_qk = np.concatenate([wq[:, hs], wk[:, hs]], axis=1)
        bqk_t = np.concatenate([bq[hs], bk[hs]]).reshape(4, 128).T.copy()
        in_maps.append(dict(
            xT=np.ascontiguousarray(x[b].T).astype(ml_dtypes.bfloat16),
            w_qk=w_qk.astype(ml_dtypes.bfloat16),
            w_v=wv[:, hs].astype(ml_dtypes.bfloat16),
            w_p=np.ascontiguousarray(Wp[hs, :]).astype(ml_dtypes.bfloat16),
            bqk=bqk_t.astype(np.float32),
            cmask=cm,
        ))
    return in_maps


def unshard(results, c_attn_b, c_proj_w, c_proj_b):
    bqkv = np.asarray(c_attn_b, np.float32)
    Wp = np.asarray(c_proj_w, np.float32)
    bp = np.asarray(c_proj_b, np.float32)
    # softmax rows sum to 1, so the v-bias passes through attention unchanged:
    # out = (softmax @ xWv + bv) @ Wp + bp = sum(partials) + bv@Wp + bp
    beff = (bqkv[2 * D:] @ Wp + bp).astype(np.float32)
    full = np.zeros((B, S, D), np.float32)
    for c in range(NCORES):
        b = c // 4
        full[b] += results[c]["out"].astype(np.float32)
    full += beff
    return full


_NC = None


def kernel(**inputs):
    global _NC
    if _NC is None:
        _NC = build_nc()
    in_maps = shard_inputs(**inputs)
    res = run_bass_kernel_spmd(_NC, in_maps, core_ids=list(range(NCORES)))
    return unshard(res.results, inputs["c_attn_b"], inputs["c_proj_w"],
                   inputs["c_proj_b"])


if __name__ == "__main__":
    import jax
    with jax.default_device(jax.devices("cpu")[0]):
        import reference
        inputs = {k: np.asarray(v) for k, v in reference.setup_inputs().items()}
        expected = np.asarray(reference.reference(**inputs))
    actual = kernel(**inputs)
    err = np.abs(actual - expected)
    print("max abs err:", err.max(), "rel:", err.max() / np.abs(expected).max())


# revision 33
# speedup vs baseline: 1.0389x; 1.0107x over previous
"""Trainium2 Bass kernel for GPT-2 style attention block (B=2, S=2048, D=1024, H=16).

Sharding (8 cores): data-parallel over batch (2) x tensor-parallel over heads (4 per
core). Each core: QKV projection for its 4 heads over the full sequence, full-seq
causal attention (transposed-scores layout: softmax reduction folded into the PV
matmul via a ones-column in V), then a row-parallel partial c_proj over the full
sequence using only this core's 256 rows of c_proj_w. No collectives: the host
sums the 4 per-head-group partials per batch (plus the folded v-bias term), so
each core's span is pure compute with no cross-core sync.

Schedule: work is organized in per-qt rounds (512 query columns each). The q/k
projections for query-block qt and key-block qt plus the V pieces for that key
range are emitted inside the round, so the tensor engine always has dense matmul
work to overlap the softmax exps (scalar engine) and stays HAM-warm.

Causal structure: score blocks on the diagonal are shrunk to their unmasked
query range and packed contiguously in PSUM so the exp covers no dead columns.
Masking is post-exp: GpSimd multiplies the probability tile by a 0/1 triangle
in SBUF (keeps DVE and the PSUM ports free). Softmax normalization is off-PE:
reciprocal on DVE, partition broadcast on GpSimd, fused scale-multiply on DVE.

Compute dtype bf16 (fp32 PSUM accumulation); normalization in fp32; partial
outputs shipped as fp16 to halve DMA.
"""
import sys
sys.path.insert(0, '/opt/trn_rl_repo')

import numpy as np
import ml_dtypes

import concourse.bass as bass
import concourse.mybir as mybir
import concourse.tile as tile
from concourse import bacc
from concourse.bass_utils import run_bass_kernel_spmd

B, S, D = 2, 2048, 1024
H, HD = 16, 64
NCORES = 8
HPC = H // 4          # heads per core = 4

F32 = mybir.dt.float32
F16 = mybir.dt.float16
BF16 = mybir.dt.bfloat16
ADD = mybir.AluOpType.add
MULT = mybir.AluOpType.mult
EXP = mybir.ActivationFunctionType.Exp


def _emit(nc, tc):
    xT = nc.dram_tensor("xT", [D, S], BF16, kind="ExternalInput").ap()
    w_qk = nc.dram_tensor("w_qk", [D, 512], BF16, kind="ExternalInput").ap()
    w_v = nc.dram_tensor("w_v", [D, 256], BF16, kind="ExternalInput").ap()
    w_p = nc.dram_tensor("w_p", [256, D], BF16, kind="ExternalInput").ap()
    bqk = nc.dram_tensor("bqk", [128, 4], F32, kind="ExternalInput").ap()
    cmask = nc.dram_tensor("cmask", [128, 128], BF16, kind="ExternalInput").ap()
    out = nc.dram_tensor("out", [S, D], F16, kind="ExternalOutput").ap()

    from contextlib import ExitStack
    ctx = ExitStack()
    cst = ctx.enter_context(tc.tile_pool(name="cst", bufs=1))
    pw = ctx.enter_context(tc.tile_pool(name="pw", bufs=2, space="PSUM"))
    pat = ctx.enter_context(tc.tile_pool(name="pat", bufs=2, space="PSUM"))
    psc = ctx.enter_context(tc.tile_pool(name="psc", bufs=2, space="PSUM"))
    sb = ctx.enter_context(tc.tile_pool(name="sb", bufs=4))

    # ---- resident SBUF loads, split across both HWDGE queues (sync + scalar)
    # and ordered so round 0's operands land first ----
    xT_sb = cst.tile([128, 8, S], BF16)
    wqk_sb = cst.tile([128, 8, 512], BF16)
    wv_sb = cst.tile([128, 8, 256], BF16)
    bqk_sb = cst.tile([128, 4], F32)
    cm_sb = cst.tile([128, 128], BF16)
    wp_sb = cst.tile([128, 2, D], BF16)
    xT_r = xT.rearrange("(k p) n -> p k n", p=128)
    wqk_r = w_qk.rearrange("(k p) n -> p k n", p=128)
    wv_r = w_v.rearrange("(k p) n -> p k n", p=128)
    nc.scalar.dma_start(bqk_sb[:], bqk)
    nc.scalar.dma_start(cm_sb[:], cmask)
    # round-0-critical prefix (xT qt0 + wqk) split across both queues
    for k in range(0, 8, 2):
        nc.sync.dma_start(xT_sb[:, k, 0:512], xT_r[:, k, 0:512])
        nc.sync.dma_start(wqk_sb[:, k], wqk_r[:, k])
        nc.scalar.dma_start(xT_sb[:, k + 1, 0:512], xT_r[:, k + 1, 0:512])
        nc.scalar.dma_start(wqk_sb[:, k + 1], wqk_r[:, k + 1])
    # rest: xT on sync (qt-major), v/proj weights on scalar
    for qt in range(1, 4):
        for k in range(8):
            nc.sync.dma_start(
                xT_sb[:, k, qt * 512:(qt + 1) * 512],
                xT_r[:, k, qt * 512:(qt + 1) * 512])
    for k in range(8):
        nc.scalar.dma_start(wv_sb[:, k], wv_r[:, k])
    nc.scalar.dma_start(wp_sb[:], w_p.rearrange("(k p) n -> p k n", p=128))

    # PE warmer: dependency-free junk matmuls keep the array busy during the
    # input DMAs so HAM unthrottles before real work arrives
    ones_sb = cst.tile([1, 128], BF16)
    nc.vector.memset(ones_sb[:], 1.0)
    wrow = sb.tile([1, 512], BF16, tag="wrow")
    nc.vector.memset(wrow[:], 1.0)
    warm_ps = pw.tile([128, 512], F32, tag="w", name="warm")
    for _ in range(12):
        nc.tensor.matmul(warm_ps[:], ones_sb[:], wrow[:],
                         start=True, stop=True)

    # qkT [512, 2048]: rows 0-255 = q^T (4 heads x 64, prescaled 1/8), 256-511 = k^T
    qkT_sb = cst.tile([128, 4, S], BF16)

    def qk_proj(m, qt):
        # q^T (m=0,1) / k^T (m=2,3) for one 512-column sequence block
        ps = pw.tile([128, 512], F32, tag="w", name=f"qk{m}_{qt}")
        for k in range(8):
            nc.tensor.matmul(
                ps[:], wqk_sb[:, k, m * 128:(m + 1) * 128],
                xT_sb[:, k, qt * 512:(qt + 1) * 512],
                start=(k == 0), stop=(k == 7))
        nc.vector.tensor_scalar(
            out=qkT_sb[:, m, qt * 512:(qt + 1) * 512], in0=ps[:],
            scalar1=bqk_sb[:, m:m + 1], scalar2=None, op0=ADD)

    # V with interleaved ones column: V_sb [128, 16, 4*65]
    V_sb = cst.tile([128, 16, HPC * 65], BF16)

    def v_ones():
        nc.vector.memset(
            V_sb[:].rearrange("p m (h c) -> p m h c", c=65)[:, :, :, 64:65], 1.0)

    def v_piece(m):
        ps = pw.tile([128, 512], F32, tag="w", name=f"v{m}")
        for k in range(8):
            nc.tensor.matmul(
                ps[:, :256], xT_sb[:, k, m * 128:(m + 1) * 128], wv_sb[:, k, :],
                start=(k == 0), stop=(k == 7))
        nc.vector.tensor_copy(
            out=V_sb[:, m].rearrange("p (h c) -> p h c", c=65)[:, :, 0:64],
            in_=ps[:, :256].rearrange("p (h c) -> p h c", c=64))

    attnT_sb = cst.tile([128, 2, S], BF16)

    def attend_pair(j, qt, defer_v=(), fillers=None, fast_norm=False):
        # heads 2j (partitions 0-63) and 2j+1 (64-127) interleaved: their K=64
        # score matmuls auto-derive different PE row-groups from base_partition
        # and run concurrently when adjacent in the queue. One key block per
        # step, both heads side by side in one PSUM tile -> one exp call covers
        # the pair and psc stays double-buffered. Head A sits at column 0,
        # head B at column 512: each matmul output must stay inside one PSUM
        # bank, so B's offset is fixed even when the diagonal block is shrunk.
        sub = j
        at = {0: pat.tile([128, 512], F32, tag="at", name=f"atA{j}_{qt}"),
              64: pat.tile([128, 512], F32, tag="at", name=f"atB{j}_{qt}")}
        nkb = 4 * qt + 4

        def scores_kb(kb):
            rel = max(0, kb * 128 - qt * 512)
            wb = 512 - rel
            sc = psc.tile([128, 1024], F32, tag="sc", name=f"sc{j}_{qt}_{kb}")
            for i, po in enumerate((0, 64)):
                nc.tensor.matmul(
                    sc[:, i * 512:i * 512 + wb],
                    qkT_sb[po:po + 64, 2 + sub, kb * 128:(kb + 1) * 128],
                    qkT_sb[po:po + 64, sub, qt * 512 + rel:(qt + 1) * 512],
                    start=True, stop=True)
            pt = sb.tile([128, 1024], BF16, tag="pt", name=f"pt{j}_{qt}_{kb}")
            if rel:
                # strided 3D AP: exp only the two valid wb-wide runs, skipping
                # the dead pad between head A's and head B's shrunk blocks
                nc.scalar.activation(
                    out=pt[:].rearrange("p (b c) -> p b c", c=512)[:, :, 0:wb],
                    in_=sc[:].rearrange("p (b c) -> p b c", c=512)[:, :, 0:wb],
                    func=EXP)
            else:
                nc.scalar.activation(out=pt[:, :1024], in_=sc[:, :1024],
                                     func=EXP)
            if kb * 128 >= qt * 512:  # post-exp triangle zeroing, both heads
                for i in (0, 1):
                    nc.vector.tensor_tensor(
                        pt[:, i * 512:i * 512 + 128],
                        pt[:, i * 512:i * 512 + 128], cm_sb[:], MULT)
            return pt

        def pv_kb(kb, pt):
            rel = max(0, kb * 128 - qt * 512)
            wb = 512 - rel
            for i, po in enumerate((0, 64)):
                h = 2 * j + i
                nc.tensor.matmul(
                    at[po][0:65, rel:512], V_sb[:, kb, h * 65:(h + 1) * 65],
                    pt[:, i * 512:i * 512 + wb],
                    start=(kb == 0), stop=(kb == nkb - 1))

        if defer_v:
            # round 0 only: all scores/exps go first so the first exps aren't
            # queued behind 32 cold-start V matmuls; V lands before the PVs
            pts = [scores_kb(kb) for kb in range(nkb)]
            for m in defer_v:
                v_piece(m)
            for kb in range(nkb):
                pv_kb(kb, pts[kb])
        else:
            # thread filler projection work into the exp-paced kb loop so the
            # tensor engine stays dense (and HAM-warm) while ACT chews exps
            for kb in range(nkb):
                pv_kb(kb, scores_kb(kb))
                if fillers and kb % 2 == 1:
                    fillers.pop(0)()
        # quick-release the at banks (2 fast DVE copies each), then normalize
        # off the critical path: both heads' denominators packed in one row,
        # one reciprocal (DVE), one partition broadcast (GpSimd), and two
        # partition-aligned in-place multiplies on SBUF (DVE). fast_norm (for
        # the final pair, where the PE is idle and the c_proj tail waits on
        # attnT) broadcasts via a low-latency PE matmul instead of GpSimd.
        den2 = sb.tile([1, 1024], F32, tag="den2")
        for po in (0, 64):
            sl = attnT_sb[po:po + 64, sub, qt * 512:(qt + 1) * 512]
            nc.vector.tensor_copy(out=sl, in_=at[po][0:64, :])
            nc.vector.tensor_copy(out=den2[:, po * 8:po * 8 + 512],
                                  in_=at[po][64:65, :])
        rec2 = sb.tile([1, 1024], F32, tag="rec2")
        nc.vector.reciprocal_approx_fast(rec2[:], den2[:])
        if fast_norm:
            rec2b = sb.tile([1, 1024], BF16, tag="rec2b")
            nc.vector.tensor_copy(out=rec2b[:], in_=rec2[:])
            bc = {po: pw.tile([128, 512], F32, tag="w", name=f"bc{j}_{qt}_{po}")
                  for po in (0, 64)}
            for po in (0, 64):
                nc.tensor.matmul(bc[po][:], ones_sb[:],
                                 rec2b[:, po * 8:po * 8 + 512],
                                 start=True, stop=True)
                sl = attnT_sb[po:po + 64, sub, qt * 512:(qt + 1) * 512]
                nc.vector.tensor_tensor(
                    sl, sl, bc[po][po:po + 64, :], MULT)
        else:
            recb = sb.tile([128, 1024], F32, tag="recb")
            nc.gpsimd.partition_broadcast(recb[:], rec2[:])
            for po in (0, 64):
                sl = attnT_sb[po:po + 64, sub, qt * 512:(qt + 1) * 512]
                nc.vector.tensor_tensor(
                    sl, sl, recb[po:po + 64, po * 8:po * 8 + 512], MULT)

    def c_proj(ms, tail=False):
        # partial c_proj: contract only this core's 256 D-rows (2 u-blocks of
        # 128), full 2048-seq output; host sums partials across head groups.
        # In the tail (scalar engine idle) the PSUM evacuation is split across
        # Vector and Scalar and the out-DMA goes on the second queue.
        for m in ms:
            out_sb = sb.tile([128, D], F16, tag="out")
            ps = [pw.tile([128, 512], F32, tag="w", name=f"pj{m}_{n}") for n in range(2)]
            for u in range(2):
                for n in range(2):
                    nc.tensor.matmul(
                        ps[n][:], attnT_sb[:, u, m * 128:(m + 1) * 128],
                        wp_sb[:, u, n * 512:(n + 1) * 512],
                        start=(u == 0), stop=(u == 1))
            nc.vector.tensor_copy(out=out_sb[:, 0:512], in_=ps[0][:])
            if tail:
                nc.scalar.activation(out=out_sb[:, 512:1024], in_=ps[1][:],
                                     func=mybir.ActivationFunctionType.Copy)
                nc.scalar.dma_start(out[m * 128:(m + 1) * 128, :], out_sb[:])
            else:
                nc.vector.tensor_copy(out=out_sb[:, 512:1024], in_=ps[1][:])
                nc.sync.dma_start(out[m * 128:(m + 1) * 128, :], out_sb[:])

    # ---- per-qt rounds, software-pipelined: the next round's q/k projections
    # and V pieces are emitted before the current round ends so the scalar
    # engine's exp stream never drains at round boundaries; prev-round c_proj
    # blocks fill exp-gated stretches ----
    v_ones()
    qk_proj(0, 0)
    qk_proj(2, 0)
    attend_pair(0, 0, defer_v=(0, 1, 2, 3))
    qk_proj(1, 0)
    qk_proj(3, 0)
    attend_pair(1, 0)
    qk_proj(0, 1)
    qk_proj(2, 1)
    for m in range(4, 8):
        v_piece(m)
    # fillers drain inside the kb loops (at odd kb steps); leftovers are
    # emitted right after, always before any consumer pair. c_proj blocks are
    # weighted toward round 3 where no projection work remains, and two are
    # held past the last pair to cover its normalize latency.
    filler_plan = {
        (1, 0): [lambda: qk_proj(1, 1), lambda: qk_proj(3, 1)],
        (1, 1): [lambda: qk_proj(0, 2), lambda: qk_proj(2, 2)]
                + [lambda m=m: v_piece(m) for m in range(8, 12)]
                + [lambda: c_proj((0,)), lambda: c_proj((1,))],
        (2, 0): [lambda: qk_proj(1, 2), lambda: qk_proj(3, 2),
                 lambda: c_proj((2,)), lambda: c_proj((3,))],
        (2, 1): [lambda: qk_proj(0, 3), lambda: qk_proj(2, 3),
                 lambda: c_proj((4,)), lambda: c_proj((5,)),
                 lambda: c_proj((6,))],
        (3, 0): [lambda m=m: v_piece(m) for m in range(12, 16)]
                + [lambda: qk_proj(1, 3), lambda: qk_proj(3, 3),
                   lambda: c_proj((7,)), lambda: c_proj((8,))],
        (3, 1): [lambda: c_proj((9,)), lambda: c_proj((10,))],
    }
    for qt in range(1, 4):
        for j in (0, 1):
            f = filler_plan[(qt, j)]
            attend_pair(j, qt, fillers=f, fast_norm=(qt == 3 and j == 1))
            for fn in f:
                fn()
    c_proj((11,))
    c_proj(tuple(range(12, 16)), tail=True)

    ctx.close()


def build_nc():
    nc = bacc.Bacc("TRN2", target_bir_lowering=False, debug=False, num_devices=NCORES)
    with tile.TileContext(nc) as tc:
        _emit(nc, tc)
    nc.compile()
    return nc


def shard_inputs(hidden_states, c_attn_w, c_attn_b, c_proj_w, c_proj_b):
    x = np.asarray(hidden_states, np.float32)
    W = np.asarray(c_attn_w, np.float32)
    bqkv = np.asarray(c_attn_b, np.float32)
    Wp = np.asarray(c_proj_w, np.float32)

    wq, wk, wv = W[:, :D] * 0.125, W[:, D:2 * D], W[:, 2 * D:]
    bq, bk = bqkv[:D] * 0.125, bqkv[D:2 * D]

    # 128x128 causal triangle keep-mask: 0 where key (row) > query (col), else 1
    k_i = np.arange(128)[:, None]
    q_i = np.arange(128)[None, :]
    cm = (k_i <= q_i).astype(ml_dtypes.bfloat16)

    in_maps = []
    for c in range(NCORES):
        b, r = divmod(c, 4)
        hs = slice(256 * r, 256 * (r + 1))
        w_qk = np.concatenate([wq[:, hs], wk[:, hs]], axis=1)
        bqk_t = np.concatenate([bq[hs], bk[hs]]).reshape(4, 128).T.copy()
        in_maps.append(dict(
            xT=np.ascontiguousarray(x[b].T).astype(ml_dtypes.bfloat16),
            w_qk=w_qk.astype(ml_dtypes.bfloat16),
            w_v=wv[:, hs].astype(ml_dtypes.bfloat16),
            w_p=np.ascontiguousarray(Wp[hs, :]).astype(ml_dtypes.bfloat16),
            bqk=bqk_t.astype(np.float32),
            cmask=cm,
        ))
    return in_maps


def unshard(results, c_attn_b, c_proj_w, c_proj_b):
    bqkv = np.asarray(c_attn_b, np.float32)
    Wp = np.asarray(c_proj_w, np.float32)
    bp = np.asarray(c_proj_b, np.float32)
    # softmax rows sum to 1, so the v-bias passes through attention unchanged:
    # out = (softmax @ xWv + bv) @ Wp + bp = sum(partials) + bv@Wp + bp
    beff = (bqkv[2 * D:] @ Wp + bp).astype(np.float32)
    full = np.zeros((B, S, D), np.float32)
    for c in range(NCORES):
        b = c // 4
        full[b] += results[c]["out"].astype(np.float32)
    full += beff
    return full


_NC = None


def kernel(**inputs):
    global _NC
    if _NC is None:
        _NC = build_nc()
    in_maps = shard_inputs(**inputs)
    res = run_bass_kernel_spmd(_NC, in_maps, core_ids=list(range(NCORES)))
    return unshard(res.results, inputs["c_attn_b"], inputs["c_proj_w"],
                   inputs["c_proj_b"])


if __name__ == "__main__":
    import jax
    with jax.default_device(jax.devices("cpu")[0]):
        import reference
        inputs = {k: np.asarray(v) for k, v in reference.setup_inputs().items()}
        expected = np.asarray(reference.reference(**inputs))
    actual = kernel(**inputs)
    err = np.abs(actual - expected)
    print("max abs err:", err.max(), "rel:", err.max() / np.abs(expected).max())
